# revision 2
# baseline (speedup 1.0000x reference)
"""Trainium2 Bass kernel for DigitConvolutionalModel.

Model: x[B,784] -> 3x3 valid conv (1 channel) -> flatten(676) -> FC(128)+relu
       -> FC(128)+relu (same W2 twice) -> FC(10).

Strategy:
  * The conv is a linear map, so conv(x)@W1 == x @ (C@W1) where C is the
    [784,676] conv operator. We fold conv_w into W1 on the host into a
    dense [784,128] matrix W1f. The whole network is then 4 dense layers.
  * Pure data parallel: batch 65536 split as 8192 per NeuronCore, weights
    replicated.
  * On-chip layout keeps activations transposed: tiles are
    [hid=128 partitions, batch free dim], so every layer is
    out = lhsT.T @ rhs with lhsT = weights (natural [in,out] layout) and
    rhs = previous activation. Only x needs a host-side transpose/retile
    (done once on host, off the device critical path).
  * bf16 operands, fp32 PSUM accumulation (rel err ~4e-3, input DMA bytes
    halved: 12.8MB/core, balancing ~35us of PE work => "ridge" regime).
  * Input streamed in chunks on both HWDGE rings (sync+scalar) so compute
    starts early and DMA overlaps compute; relu+bias fused into ScalarE
    activation (L1/L3) and VectorE tensor_scalar (L2/L4).
"""

import os
import sys

sys.path.insert(0, "/opt/trn_rl_repo")

import ml_dtypes
import numpy as np

import concourse.bacc as bacc
import concourse.mybir as mybir
import concourse.tile as tile
from concourse.bass_utils import run_bass_kernel_spmd

B = 65536
IN_SIDE = 28
KSZ = 3
OUT_SIDE = IN_SIDE - KSZ + 1  # 26
FLAT = OUT_SIDE * OUT_SIDE  # 676
IN_FLAT = IN_SIDE * IN_SIDE  # 784
HID = 128
OUT = 10

N_CORES = 8
B_SHARD = B // N_CORES  # 8192
KP = 112  # feature-tile partition size (784 = 7*112, uniform tiles)
KT = IN_FLAT // KP  # 7
DMA_J = int(os.environ.get("KERNEL_DMA_J", "512"))  # batch cols per input DMA
MM_J = 512  # batch columns per matmul (one fp32 PSUM bank)
X_BUFS = int(os.environ.get("KERNEL_X_BUFS", "8"))
H_BUFS = int(os.environ.get("KERNEL_H_BUFS", "4"))
PS_BUFS = int(os.environ.get("KERNEL_PS_BUFS", "8"))

BF16 = mybir.dt.bfloat16
F32 = mybir.dt.float32

LAST_EXEC_NS = None
LAST_RESULTS = None

_compiled = {}


def _build_program():
    n_chunks = B_SHARD // DMA_J
    sub = DMA_J // MM_J  # compute sub-chunks per DMA chunk

    nc = bacc.Bacc(
        "TRN2", target_bir_lowering=False, debug=False, num_devices=N_CORES
    )
    # host-retiled input: chunk c holds [KP, KT*DMA_J] contiguous
    xt = nc.dram_tensor("xt", [n_chunks, KP, KT * DMA_J], BF16, kind="ExternalInput")
    w1 = nc.dram_tensor("w1", [IN_FLAT, HID], BF16, kind="ExternalInput")
    w2 = nc.dram_tensor("w2", [HID, HID], BF16, kind="ExternalInput")
    w3 = nc.dram_tensor("w3", [HID, OUT], BF16, kind="ExternalInput")
    b1 = nc.dram_tensor("b1", [HID, 1], F32, kind="ExternalInput")
    b2 = nc.dram_tensor("b2", [HID, 1], F32, kind="ExternalInput")
    b3 = nc.dram_tensor("b3", [OUT, 1], F32, kind="ExternalInput")
    yt = nc.dram_tensor("yt", [OUT, B_SHARD], F32, kind="ExternalOutput")

    w13 = w1.ap().rearrange("(k p) m -> p k m", p=KP)

    Relu = mybir.ActivationFunctionType.Relu
    add = mybir.AluOpType.add
    amax = mybir.AluOpType.max

    with tile.TileContext(nc) as tc:
        with (
            tc.tile_pool(name="wpool", bufs=1) as wpool,
            tc.tile_pool(name="xpool", bufs=X_BUFS) as xpool,
            tc.tile_pool(name="hpool", bufs=H_BUFS) as hpool,
            tc.tile_pool(name="opool", bufs=1) as opool,
            tc.tile_pool(name="psum", bufs=PS_BUFS, space="PSUM") as pp,
        ):
            # weights via SWDGE (gpsimd) to keep HWDGE rings free for x
            w1_sb = wpool.tile([KP, KT, HID], BF16)
            nc.gpsimd.dma_start(out=w1_sb[:], in_=w13)
            w2_sb = wpool.tile([HID, HID], BF16)
            nc.gpsimd.dma_start(out=w2_sb[:], in_=w2.ap())
            w3_sb = wpool.tile([HID, OUT], BF16)
            nc.gpsimd.dma_start(out=w3_sb[:], in_=w3.ap())
            b1_sb = wpool.tile([HID, 1], F32)
            nc.gpsimd.dma_start(out=b1_sb[:], in_=b1.ap())
            b2_sb = wpool.tile([HID, 1], F32)
            nc.gpsimd.dma_start(out=b2_sb[:], in_=b2.ap())
            b3_sb = wpool.tile([OUT, 1], F32)
            nc.gpsimd.dma_start(out=b3_sb[:], in_=b3.ap())

            yt_sb = opool.tile([OUT, B_SHARD], F32)
            out_flushed = 0

            for c in range(n_chunks):
                xt_sb = xpool.tile([KP, KT * DMA_J], BF16, tag="xt")
                eng = nc.sync if c % 2 == 0 else nc.scalar
                eng.dma_start(out=xt_sb[:], in_=xt.ap()[c, :, :])
                for s in range(sub):
                    # L1: h1 = relu(W1f.T @ xT + b1)
                    ps1 = pp.tile([HID, MM_J], F32, tag="ps")
                    for k in range(KT):
                        lo = k * DMA_J + s * MM_J
                        nc.tensor.matmul(
                            ps1[:],
                            w1_sb[:, k, :],
                            xt_sb[:, lo : lo + MM_J],
                            start=(k == 0),
                            stop=(k == KT - 1),
                        )
                    h1 = hpool.tile([HID, MM_J], BF16, tag="h1")
                    nc.scalar.activation(h1[:], ps1[:], Relu, bias=b1_sb[:])
                    # L2: h2 = relu(W2.T @ h1 + b2)   (VectorE)
                    ps2 = pp.tile([HID, MM_J], F32, tag="ps")
                    nc.tensor.matmul(ps2[:], w2_sb[:], h1[:], start=True, stop=True)
                    h2 = hpool.tile([HID, MM_J], BF16, tag="h2")
                    nc.vector.tensor_scalar(
                        out=h2[:],
                        in0=ps2[:],
                        scalar1=b2_sb[:],
                        scalar2=0.0,
                        op0=add,
                        op1=amax,
                    )
                    # L3: h3 = relu(W2.T @ h2 + b2)   (ScalarE)
                    ps3 = pp.tile([HID, MM_J], F32, tag="ps")
                    nc.tensor.matmul(ps3[:], w2_sb[:], h2[:], start=True, stop=True)
                    h3 = hpool.tile([HID, MM_J], BF16, tag="h3")
                    nc.scalar.activation(h3[:], ps3[:], Relu, bias=b2_sb[:])
                    # L4: y = W3.T @ h3 + b3          (VectorE)
                    ps4 = pp.tile([OUT, MM_J], F32, tag="ps")
                    nc.tensor.matmul(ps4[:], w3_sb[:], h3[:], start=True, stop=True)
                    j0 = c * DMA_J + s * MM_J
                    nc.vector.tensor_scalar(
                        out=yt_sb[:, j0 : j0 + MM_J],
                        in0=ps4[:],
                        scalar1=b3_sb[:],
                        scalar2=None,
                        op0=add,
                    )
                # flush finished output every ~2048 batch cols (SWDGE ring)
                done = (c + 1) * DMA_J
                if done - out_flushed >= 2048 or c == n_chunks - 1:
                    nc.gpsimd.dma_start(
                        out=yt.ap()[:, out_flushed:done],
                        in_=yt_sb[:, out_flushed:done],
                    )
                    out_flushed = done

    nc.compile()
    return nc


def _fold_conv_into_w1(conv_w, W1):
    """W1f[784,128] such that x @ W1f == conv(x).flatten @ W1."""
    W1_img = np.asarray(W1, np.float64).reshape(OUT_SIDE, OUT_SIDE, HID)
    cw = np.asarray(conv_w, np.float64).reshape(KSZ, KSZ)
    W1f = np.zeros((IN_SIDE, IN_SIDE, HID), np.float64)
    for di in range(KSZ):
        for dj in range(KSZ):
            W1f[di : di + OUT_SIDE, dj : dj + OUT_SIDE, :] += cw[di, dj] * W1_img
    return W1f.reshape(IN_FLAT, HID)


def _retile_shard(shard_bf):
    """[B_SHARD, 784] bf16 -> [n_chunks, KP, KT*DMA_J] with
    element (c, p, k*DMA_J + j) = x[c*DMA_J + j, k*KP + p]."""
    n_chunks = B_SHARD // DMA_J
    # [B_SHARD, 784] -> [n_chunks, DMA_J, KT, KP] -> [n_chunks, KP, KT, DMA_J]
    v = shard_bf.reshape(n_chunks, DMA_J, KT, KP)
    return np.ascontiguousarray(v.transpose(0, 3, 2, 1)).reshape(
        n_chunks, KP, KT * DMA_J
    )


def kernel(x, conv_w, W1, b1, W2, b2, W3, b3):
    global LAST_EXEC_NS, LAST_RESULTS
    x = np.asarray(x)
    W1f = _fold_conv_into_w1(conv_w, W1)

    bf = ml_dtypes.bfloat16
    w1_np = W1f.astype(bf)
    w2_np = np.asarray(W2, np.float32).astype(bf)
    w3_np = np.asarray(W3, np.float32).astype(bf)
    b1_np = np.asarray(b1, np.float32).reshape(HID, 1)
    b2_np = np.asarray(b2, np.float32).reshape(HID, 1)
    b3_np = np.asarray(b3, np.float32).reshape(OUT, 1)

    if "prog" not in _compiled:
        _compiled["prog"] = _build_program()
    nc = _compiled["prog"]

    in_maps = []
    for c in range(N_CORES):
        shard = x[c * B_SHARD : (c + 1) * B_SHARD, :].astype(bf)
        in_maps.append(
            {
                "xt": _retile_shard(shard),
                "w1": w1_np,
                "w2": w2_np,
                "w3": w3_np,
                "b1": b1_np,
                "b2": b2_np,
                "b3": b3_np,
            }
        )

    trace = bool(int(os.environ.get("KERNEL_TRACE", "0")))
    res = run_bass_kernel_spmd(
        nc, in_maps, core_ids=list(range(N_CORES)), trace=trace
    )
    LAST_EXEC_NS = res.exec_time_ns
    LAST_RESULTS = res

    out = np.empty((B, OUT), np.float32)
    for c in range(N_CORES):
        out[c * B_SHARD : (c + 1) * B_SHARD, :] = res.results[c]["yt"].T
    return out
